# revision 11
# baseline (speedup 1.0000x reference)
"""Trainium2 Bass kernel for a 2-layer GAT (PyG-style) over a random graph.

Strategy (8 NeuronCores, SPMD single program):
  - Destination-partition the 100k nodes contiguously across 8 cores
    (12500 each).  Within a core, sort nodes by in-degree and process
    them in tiles of 128 (one node per SBUF partition), padding each
    tile's edge count to the tile max degree.  Tiles are batched into
    groups of equal padded degree with <=128 edge slots so vector-engine
    instruction overheads amortize.
  - Per layer, a per-node "record" table (features + attention dot
    products, fp16) is built on-device, sharded, and AllGathered so every
    core holds the full table in its DRAM.  Edge messages are fetched
    with indirect DMA (one 128-row gather instruction per padded-degree
    slot column; offsets are int32 table-row ids).
  - Softmax normalization is deferred: accumulate sum(exp(e)) and
    sum(exp(e) * h_src) per node, divide once per node.  Layer 2 uses
    linearity to aggregate h2 (64ch) and apply W2 after aggregation.
  - Layer-1 output (post-ELU) plus layer-2 attention scalars form the
    layer-2 record table directly; a second AllGather shares it.

The kernel function accepts the FULL inputs and returns the FULL output.
"""

import sys

for _p in ("/opt/trn_rl_repo",):
    if _p not in sys.path:
        sys.path.insert(0, _p)

import numpy as np

# ---------------------------------------------------------------- constants
N = 100000
E = 3200000
F_IN = 128
H1 = 8           # layer-1 heads
C1 = 8           # layer-1 channels per head
HC1 = H1 * C1    # 64
C2 = 40          # layer-2 channels (1 head)
NCORES = 8
NPC = N // NCORES            # 12500 nodes per core
TILES = (NPC + 127) // 128   # 98
NPAD = TILES * 128           # 12544
SHARD = 12800                # table rows per core (25 * 512), >= NPAD
REC1 = 80                    # fp16 words: h1[64] | a_src1[8] | a_dst1[8]
REC2 = 66                    # fp16 words: h2[64] | a_src2[1] | a_dst2[1]
GREC1 = 72                   # gathered slot words (h1 | a_src1; a_dst unused)
GREC2 = 65                   # gathered slot words (h2 | a_src2; a_dst unused)
NEG_SLOPE = 0.2
EPS = 1e-16
PAD_LOGIT = -30000.0         # a_src for pad rows -> exp == 0
GROUP_SLOTS = 128            # max padded edge slots per tile group
DEBUG_BUILD = False          # add intermediate-table debug outputs


# ---------------------------------------------------------------- host prep
def _preprocess(edge_index):
    """Build the per-core tile schedule and gather-offset tables.

    Self-loop edges (PyG adds one per node) are EXCLUDED from the gather
    schedule; the kernel adds the self-loop contribution directly from the
    on-chip `own` record tile instead, saving one gather column per tile.
    """
    src = np.ascontiguousarray(edge_index[0]).astype(np.int64)
    dst = np.ascontiguousarray(edge_index[1]).astype(np.int64)
    # NOTE: the PyG-added per-node self-loop is NOT concatenated here; the
    # kernel adds that term from the on-chip own-record tile. Explicit
    # self-loops already present in edge_index stay in the gather schedule.

    deg = np.bincount(dst, minlength=N)

    perms = np.empty((NCORES, NPC), np.int64)   # node ids, degree-desc per core
    pos = np.empty(N, np.int64)                 # position of node in its core perm
    for c in range(NCORES):
        nodes = np.arange(c * NPC, (c + 1) * NPC)
        order = np.argsort(-deg[nodes], kind="stable")
        perm = nodes[order]
        perms[c] = perm
        pos[perm] = np.arange(NPC)

    row_of = (np.arange(N) // NPC) * SHARD + pos    # table row of each node

    # common tile degree schedule (max over cores per tile index)
    degs_sorted = np.zeros((NCORES, NPAD), np.int64)
    degs_sorted[:, :NPC] = deg[perms]
    dtile = degs_sorted.reshape(NCORES, TILES, 128).max(axis=2).max(axis=0)
    dtile = np.maximum(dtile, 1)

    # group consecutive tiles: equal padded degree, <= GROUP_SLOTS slots.
    # Only merge tiles whose max degree matches the group's exactly, so
    # grouping adds no extra gather columns (each column is one indirect
    # DMA instruction -- the kernel's bottleneck).
    groups = []  # (tile_start, ntiles, dbar)
    t = 0
    while t < TILES:
        dbar = int(dtile[t])
        jmax = min(max(1, GROUP_SLOTS // dbar), TILES - t)
        j = 1
        while j < jmax and dtile[t + j] >= dbar:
            j += 1
        groups.append((t, j, dbar))
        t += j

    # per-tile padded degree after grouping and column offsets
    dpad = np.empty(TILES, np.int64)
    for (t0, j, dbar) in groups:
        dpad[t0:t0 + j] = dbar
    col0 = np.zeros(TILES + 1, np.int64)
    np.cumsum(dpad, out=col0[1:])
    sd = int(col0[-1])

    # offsets[c, p, col]: table row gathered into slot (tile, p, d)
    offs = np.empty((NCORES, 128, sd), np.int32)
    for c in range(NCORES):
        offs[c] = c * SHARD + (SHARD - 1)   # pad row (a_src == PAD_LOGIT)

    order_e = np.argsort(dst, kind="stable")
    ds = dst[order_e]
    sv = src[order_e]
    counts = np.bincount(ds, minlength=N)
    seg_start = np.concatenate([[0], np.cumsum(counts)[:-1]])
    rank = np.arange(ds.shape[0]) - seg_start[ds]

    c_e = ds // NPC
    pos_e = pos[ds]
    p_e = pos_e % 128
    t_e = pos_e // 128
    col_e = col0[t_e] + rank
    offs[c_e, p_e, col_e] = row_of[sv].astype(np.int32)

    return dict(perms=perms, dtile=dpad, groups=groups, col0=col0, sd=sd,
                offs=offs, deg=deg)


def _make_inputs(x, W1, att_src1, att_dst1, b1, W2, att_src2, att_dst2, b2, prep):
    """Per-core input maps for the SPMD kernel."""
    f16 = np.float16
    f32 = np.float32

    # layer-1 attention folded into x-side weights: a1 = x @ (W1 @ Atil)
    Atil = np.zeros((HC1, 16), f32)
    for h in range(H1):
        Atil[h * C1:(h + 1) * C1, h] = att_src1[h]
        Atil[h * C1:(h + 1) * C1, 8 + h] = att_dst1[h]
    wa1 = (W1.astype(f32) @ Atil).astype(f16)               # [128, 16]

    ws2 = (W2.astype(f32) @ att_src2[0].astype(f32)).astype(f16)  # [64]
    wd2 = (W2.astype(f32) @ att_dst2[0].astype(f32)).astype(f16)  # [64]

    common = {
        "w1": W1.astype(f16),                               # [128, 64]
        "wa1": wa1,                                         # [128, 16]
        "ws2r": np.tile(ws2, (128, 1)),                     # [128, 64] fp16
        "wd2r": np.tile(wd2, (128, 1)),                     # [128, 64] fp16
        "w2": W2.astype(f32),                               # [64, 40]
        "b1r": np.tile(b1.astype(f32), (128, 1)),           # [128, 64]
        "b2r": np.tile(b2.astype(f32), (128, 1)),           # [128, 40]
    }
    pr1 = np.zeros((128, REC1), f16)
    pr1[:, 64:72] = PAD_LOGIT
    pr2 = np.zeros((128, REC2), f16)
    pr2[:, 64] = PAD_LOGIT
    common["padrec1"] = pr1
    common["padrec2"] = pr2

    in_maps = []
    for c in range(NCORES):
        xt = np.zeros((SHARD, F_IN), f16)
        xt[:NPC] = x[prep["perms"][c]].astype(f16)
        m = dict(common)
        m["xts"] = np.ascontiguousarray(xt.T)               # [128, SHARD] fp16
        m["offs"] = prep["offs"][c]                         # [128, sd] int32
        in_maps.append(m)
    return in_maps


# ---------------------------------------------------------------- bass build
def _build(prep):
    from concourse import bass, bacc, mybir
    from concourse.tile import TileContext
    from concourse.masks import make_identity

    f16 = mybir.dt.float16
    f32 = mybir.dt.float32
    i32 = mybir.dt.int32
    OP = mybir.AluOpType
    ACTF = mybir.ActivationFunctionType
    AX = mybir.AxisListType

    groups = prep["groups"]
    col0 = prep["col0"]
    sd = prep["sd"]

    nc = bacc.Bacc("TRN2", target_bir_lowering=False, debug=False,
                   num_devices=NCORES)

    xts = nc.declare_dram_parameter("xts", [128, SHARD], f16, isOutput=False)
    offs = nc.declare_dram_parameter("offs", [128, sd], i32, isOutput=False)
    w1 = nc.declare_dram_parameter("w1", [128, HC1], f16, isOutput=False)
    wa1 = nc.declare_dram_parameter("wa1", [128, 16], f16, isOutput=False)
    ws2r = nc.declare_dram_parameter("ws2r", [128, HC1], f16, isOutput=False)
    wd2r = nc.declare_dram_parameter("wd2r", [128, HC1], f16, isOutput=False)
    w2 = nc.declare_dram_parameter("w2", [HC1, C2], f32, isOutput=False)
    b1r = nc.declare_dram_parameter("b1r", [128, HC1], f32, isOutput=False)
    b2r = nc.declare_dram_parameter("b2r", [128, C2], f32, isOutput=False)
    padrec1 = nc.declare_dram_parameter("padrec1", [128, REC1], f16, isOutput=False)
    padrec2 = nc.declare_dram_parameter("padrec2", [128, REC2], f16, isOutput=False)
    outp = nc.declare_dram_parameter("out", [NPAD, C2], f32, isOutput=True)
    if DEBUG_BUILD:
        d_bnc1 = nc.declare_dram_parameter("d_bnc1", [SHARD, REC1], f16,
                                           isOutput=True)
        d_bnc2 = nc.declare_dram_parameter("d_bnc2", [SHARD, REC2], f16,
                                           isOutput=True)
        d_tab1 = nc.declare_dram_parameter("d_tab1", [NCORES * SHARD, REC1], f16,
                                           isOutput=True)
        S0 = groups[0][1] * groups[0][2]
        d_g = nc.declare_dram_parameter("d_g", [128, S0 * REC1], f16,
                                        isOutput=True)
        d_e = nc.declare_dram_parameter("d_e", [128, S0 * H1], f32,
                                        isOutput=True)
        d_ex = nc.declare_dram_parameter("d_ex", [128, S0 * H1], f32,
                                         isOutput=True)
        d_den = nc.declare_dram_parameter("d_den", [128, groups[0][1] * H1], f32,
                                          isOutput=True)
        d_ms = nc.declare_dram_parameter("d_ms", [128, groups[0][1] * HC1], f32,
                                         isOutput=True)
        d_o1 = nc.declare_dram_parameter("d_o1", [128, groups[0][1] * HC1], f32,
                                         isOutput=True)

    bnc1 = nc.dram_tensor("bounce1", [SHARD, REC1], f16)
    tab1 = nc.dram_tensor("table1", [NCORES * SHARD, REC1], f16, addr_space="Shared")
    bnc2 = nc.dram_tensor("bounce2", [SHARD, REC2], f16)
    tab2 = nc.dram_tensor("table2", [NCORES * SHARD, REC2], f16, addr_space="Shared")

    with TileContext(nc) as tc:
        with (
            tc.tile_pool(name="const", bufs=1) as cpool,
            tc.tile_pool(name="dense", bufs=3) as dpool,
            tc.tile_pool(name="gth", bufs=2) as gpool,
            tc.tile_pool(name="mbuf", bufs=2) as mpool,
            tc.tile_pool(name="small", bufs=3) as spool,
            tc.tile_pool(name="psum", bufs=2, space="PSUM") as ppool,
        ):
            # ---- resident constants
            w1s = cpool.tile([128, HC1], f16)
            nc.sync.dma_start(out=w1s[:], in_=w1[:])
            wa1s = cpool.tile([128, 16], f16)
            nc.sync.dma_start(out=wa1s[:], in_=wa1[:])
            ws2s = cpool.tile([128, HC1], f16)
            nc.sync.dma_start(out=ws2s[:], in_=ws2r[:])
            wd2s = cpool.tile([128, HC1], f16)
            nc.sync.dma_start(out=wd2s[:], in_=wd2r[:])
            w2s = cpool.tile([HC1, C2], f32)
            nc.sync.dma_start(out=w2s[:], in_=w2[:])
            b1s = cpool.tile([128, HC1], f32)
            nc.sync.dma_start(out=b1s[:], in_=b1r[:])
            b2s = cpool.tile([128, C2], f32)
            nc.sync.dma_start(out=b2s[:], in_=b2r[:])
            pr1s = cpool.tile([128, REC1], f16)
            nc.sync.dma_start(out=pr1s[:], in_=padrec1[:])
            pr2s = cpool.tile([128, REC2], f16)
            nc.sync.dma_start(out=pr2s[:], in_=padrec2[:])
            idn = cpool.tile([128, 128], f32)
            make_identity(nc, idn[:])


            # ---- dense phase: per-node records for layer 1 (own shard)
            for k in range(SHARD // 512):
                xt = dpool.tile([128, 512], f16, tag="xt")
                nc.sync.dma_start(out=xt[:], in_=xts[:, k * 512:(k + 1) * 512])
                for j in range(4):
                    ps = ppool.tile([128, REC1], f32, tag="psd")
                    lhs = xt[:, j * 128:(j + 1) * 128]
                    nc.tensor.matmul(ps[:, 0:HC1], lhsT=lhs, rhs=w1s[:],
                                     start=True, stop=True)
                    nc.tensor.matmul(ps[:, HC1:HC1 + 16], lhsT=lhs, rhs=wa1s[:],
                                     start=True, stop=True)
                    rec = dpool.tile([128, REC1], f16, tag="rec")
                    nc.vector.tensor_copy(out=rec[:], in_=ps[:])
                    r0 = (k * 4 + j) * 128
                    nc.sync.dma_start(out=bnc1[r0:r0 + 128, :], in_=rec[:])
            # pad-row tail [NPAD, SHARD): overwrite after the dense loop
            for i in range((SHARD - NPAD) // 128):
                r0 = NPAD + i * 128
                nc.sync.dma_start(out=bnc1[r0:r0 + 128, :], in_=pr1s[:])
                nc.sync.dma_start(out=bnc2[r0:r0 + 128, :], in_=pr2s[:])

            # offset table loads here so it overlaps the dense phase and
            # AllGather instead of delaying the first dense x-tile DMA
            # (HWDGE DMAs drain in program order per issuing engine).
            offs_sb = cpool.tile([128, sd], i32)
            nc.sync.dma_start(out=offs_sb[:], in_=offs[:])

            nc.gpsimd.collective_compute(
                "AllGather", OP.bypass,
                replica_groups=[list(range(NCORES))],
                ins=[bnc1[:]], outs=[tab1[:]],
            )

            # ---- layer 1 tile groups
            for (t0, J, D) in groups:
                S = J * D      # edge slots in group
                g = gpool.tile([128, S * GREC1], f16, tag="g")
                for s in range(S):
                    nc.gpsimd.indirect_dma_start(
                        out=g[:, s * GREC1:(s + 1) * GREC1], out_offset=None,
                        in_=tab1[:],
                        in_offset=bass.IndirectOffsetOnAxis(
                            ap=offs_sb[:, col0[t0] + s:col0[t0] + s + 1], axis=0),
                    )
                own = spool.tile([128, J * REC1], f16, tag="own")
                nc.sync.dma_start(
                    out=own[:].rearrange("p (j r) -> p j r", r=REC1),
                    in_=bnc1[t0 * 128:(t0 + J) * 128, :]
                        .rearrange("(j p) r -> p j r", p=128))

                gv = g[:].rearrange("p (s r) -> p s r", r=GREC1)
                ownv = own[:].rearrange("p (j r) -> p j r", r=REC1)

                # e = a_src[src] + a_dst[dst]  -> leaky relu -> exp
                e = spool.tile([128, S * H1], f32, tag="e")
                nc.vector.tensor_tensor(
                    out=e[:].rearrange("p (j d h) -> p j d h", j=J, h=H1),
                    in0=gv[:, :, 64:72].rearrange("p (j d) h -> p j d h", j=J),
                    in1=ownv[:, :, 72:80].unsqueeze(2).broadcast_to([128, J, D, H1]),
                    op=OP.add)
                el = spool.tile([128, S * H1], f32, tag="el")
                nc.vector.scalar_tensor_tensor(
                    out=el[:], in0=e[:], scalar=NEG_SLOPE, in1=e[:],
                    op0=OP.mult, op1=OP.max)
                ex = spool.tile([128, S * H1], f32, tag="ex")
                nc.scalar.activation(out=ex[:], in_=el[:], func=ACTF.Exp)

                den = spool.tile([128, J * H1], f32, tag="den")
                nc.vector.tensor_reduce(
                    out=den[:],
                    in_=ex[:].rearrange("p (j d h) -> p j h d", j=J, h=H1),
                    axis=AX.X, op=OP.add)

                m = mpool.tile([128, S * HC1], f32, tag="m")
                nc.vector.tensor_tensor(
                    out=m[:].rearrange("p (s h c) -> p s h c", h=H1, c=C1),
                    in0=gv[:, :, 0:HC1].rearrange("p s (h c) -> p s h c", h=H1),
                    in1=ex[:].rearrange("p (s h) -> p s h", h=H1)
                        .unsqueeze(3).broadcast_to([128, S, H1, C1]),
                    op=OP.mult)
                ms = spool.tile([128, J * HC1], f32, tag="ms")
                nc.vector.tensor_reduce(
                    out=ms[:],
                    in_=m[:].rearrange("p (j d c) -> p j c d", j=J, c=HC1),
                    axis=AX.X, op=OP.add)

                # self-loop term straight from the own records (not gathered)
                eo = spool.tile([128, J * H1], f32, tag="eo")
                nc.vector.tensor_tensor(
                    out=eo[:].rearrange("p (j h) -> p j h", h=H1),
                    in0=ownv[:, :, 64:72],
                    in1=ownv[:, :, 72:80],
                    op=OP.add)
                nc.vector.scalar_tensor_tensor(
                    out=eo[:], in0=eo[:], scalar=NEG_SLOPE, in1=eo[:],
                    op0=OP.mult, op1=OP.max)
                exo = spool.tile([128, J * H1], f32, tag="exo")
                nc.scalar.activation(out=exo[:], in_=eo[:], func=ACTF.Exp)
                nc.vector.tensor_tensor(out=den[:], in0=den[:], in1=exo[:],
                                        op=OP.add)
                mo = spool.tile([128, J * HC1], f32, tag="mo")
                nc.vector.tensor_tensor(
                    out=mo[:].rearrange("p (j h c) -> p j h c", h=H1, c=C1),
                    in0=ownv[:, :, 0:HC1].rearrange("p j (h c) -> p j h c", h=H1),
                    in1=exo[:].rearrange("p (j h) -> p j h", h=H1)
                        .unsqueeze(3).broadcast_to([128, J, H1, C1]),
                    op=OP.mult)
                nc.vector.tensor_tensor(out=ms[:], in0=ms[:], in1=mo[:],
                                        op=OP.add)

                rc = spool.tile([128, J * H1], f32, tag="rc")
                nc.vector.tensor_scalar_add(out=rc[:], in0=den[:], scalar1=EPS)
                nc.vector.reciprocal(out=rc[:], in_=rc[:])

                o1 = spool.tile([128, J * HC1], f32, tag="o1")
                nc.vector.tensor_tensor(
                    out=o1[:].rearrange("p (j h c) -> p j h c", h=H1, c=C1),
                    in0=ms[:].rearrange("p (j h c) -> p j h c", h=H1, c=C1),
                    in1=rc[:].rearrange("p (j h) -> p j h", h=H1)
                        .unsqueeze(3).broadcast_to([128, J, H1, C1]),
                    op=OP.mult)
                # + b1 (broadcast rows pre-replicated on host)
                nc.vector.tensor_tensor(
                    out=o1[:].rearrange("p (j c) -> p j c", c=HC1),
                    in0=o1[:].rearrange("p (j c) -> p j c", c=HC1),
                    in1=b1s[:].unsqueeze(1).broadcast_to([128, J, HC1]),
                    op=OP.add)

                if DEBUG_BUILD and t0 == 0:
                    nc.sync.dma_start(out=d_g[:], in_=g[:])
                    nc.sync.dma_start(out=d_e[:], in_=e[:])
                    nc.sync.dma_start(out=d_ex[:], in_=ex[:])
                    nc.sync.dma_start(out=d_den[:], in_=den[:])
                    nc.sync.dma_start(out=d_ms[:], in_=ms[:])
                    nc.sync.dma_start(out=d_o1[:], in_=o1[:])

                # ELU -> h2 (fp16, straight into the layer-2 record)
                t1 = spool.tile([128, J * HC1], f32, tag="t1")
                nc.vector.tensor_scalar_min(out=t1[:], in0=o1[:], scalar1=0.0)
                nc.scalar.activation(out=t1[:], in_=t1[:], func=ACTF.Exp)
                rec2 = spool.tile([128, J * REC2], f16, tag="rec2")
                r2v = rec2[:].rearrange("p (j r) -> p j r", r=REC2)
                nc.vector.scalar_tensor_tensor(
                    out=r2v[:, :, 0:HC1],
                    in0=t1[:].rearrange("p (j c) -> p j c", c=HC1),
                    scalar=-1.0,
                    in1=o1[:].rearrange("p (j c) -> p j c", c=HC1),
                    op0=OP.add, op1=OP.max)

                # a2s/a2d = h2 . (W2 @ att2)
                q = spool.tile([128, J * HC1], f32, tag="q")
                a2 = spool.tile([128, 2 * J], f32, tag="a2")
                for idx, wv in enumerate((ws2s, wd2s)):
                    nc.vector.tensor_tensor(
                        out=q[:].rearrange("p (j c) -> p j c", c=HC1),
                        in0=r2v[:, :, 0:HC1],
                        in1=wv[:].unsqueeze(1).broadcast_to([128, J, HC1]),
                        op=OP.mult)
                    nc.vector.tensor_reduce(
                        out=a2[:, idx * J:(idx + 1) * J],
                        in_=q[:].rearrange("p (j c) -> p j c", c=HC1),
                        axis=AX.X, op=OP.add)
                nc.vector.tensor_copy(out=r2v[:, :, 64:65],
                                      in_=a2[:, 0:J].unsqueeze(2))
                nc.vector.tensor_copy(out=r2v[:, :, 65:66],
                                      in_=a2[:, J:2 * J].unsqueeze(2))

                nc.sync.dma_start(
                    out=bnc2[t0 * 128:(t0 + J) * 128, :]
                        .rearrange("(j p) r -> p j r", p=128),
                    in_=rec2[:].rearrange("p (j r) -> p j r", r=REC2))

            nc.gpsimd.collective_compute(
                "AllGather", OP.bypass,
                replica_groups=[list(range(NCORES))],
                ins=[bnc2[:]], outs=[tab2[:]],
            )

            # ---- layer 2 tile groups
            for (t0, J, D) in groups:
                S = J * D
                g2 = gpool.tile([128, S * GREC2], f16, tag="g")
                for s in range(S):
                    nc.gpsimd.indirect_dma_start(
                        out=g2[:, s * GREC2:(s + 1) * GREC2], out_offset=None,
                        in_=tab2[:],
                        in_offset=bass.IndirectOffsetOnAxis(
                            ap=offs_sb[:, col0[t0] + s:col0[t0] + s + 1], axis=0),
                    )
                own2 = spool.tile([128, J * REC2], f16, tag="own")
                nc.sync.dma_start(
                    out=own2[:].rearrange("p (j r) -> p j r", r=REC2),
                    in_=bnc2[t0 * 128:(t0 + J) * 128, :]
                        .rearrange("(j p) r -> p j r", p=128))

                g2v = g2[:].rearrange("p (s r) -> p s r", r=GREC2)
                o2v = own2[:].rearrange("p (j r) -> p j r", r=REC2)

                e2 = spool.tile([128, S], f32, tag="e")
                nc.vector.tensor_tensor(
                    out=e2[:].rearrange("p (j d) -> p j d", j=J),
                    in0=g2v[:, :, 64].rearrange("p (j d) -> p j d", j=J),
                    in1=o2v[:, :, 65].unsqueeze(2).broadcast_to([128, J, D]),
                    op=OP.add)
                el2 = spool.tile([128, S], f32, tag="el")
                nc.vector.scalar_tensor_tensor(
                    out=el2[:], in0=e2[:], scalar=NEG_SLOPE, in1=e2[:],
                    op0=OP.mult, op1=OP.max)
                ex2 = spool.tile([128, S], f32, tag="ex")
                nc.scalar.activation(out=ex2[:], in_=el2[:], func=ACTF.Exp)

                den2 = spool.tile([128, J], f32, tag="den")
                nc.vector.tensor_reduce(
                    out=den2[:],
                    in_=ex2[:].rearrange("p (j d) -> p j d", j=J),
                    axis=AX.X, op=OP.add)

                m2 = mpool.tile([128, S * HC1], f32, tag="m")
                nc.vector.tensor_tensor(
                    out=m2[:].rearrange("p (s c) -> p s c", c=HC1),
                    in0=g2v[:, :, 0:HC1],
                    in1=ex2[:].unsqueeze(2).broadcast_to([128, S, HC1]),
                    op=OP.mult)
                msh = spool.tile([128, J * HC1], f32, tag="ms")
                nc.vector.tensor_reduce(
                    out=msh[:],
                    in_=m2[:].rearrange("p (j d c) -> p j c d", j=J, c=HC1),
                    axis=AX.X, op=OP.add)

                # self-loop term from the own records
                eo2 = spool.tile([128, J], f32, tag="eo")
                nc.vector.tensor_tensor(
                    out=eo2[:], in0=o2v[:, :, 64], in1=o2v[:, :, 65], op=OP.add)
                nc.vector.scalar_tensor_tensor(
                    out=eo2[:], in0=eo2[:], scalar=NEG_SLOPE, in1=eo2[:],
                    op0=OP.mult, op1=OP.max)
                exo2 = spool.tile([128, J], f32, tag="exo")
                nc.scalar.activation(out=exo2[:], in_=eo2[:], func=ACTF.Exp)
                nc.vector.tensor_tensor(out=den2[:], in0=den2[:], in1=exo2[:],
                                        op=OP.add)
                mo2 = spool.tile([128, J * HC1], f32, tag="mo")
                nc.vector.tensor_tensor(
                    out=mo2[:].rearrange("p (j c) -> p j c", c=HC1),
                    in0=o2v[:, :, 0:HC1],
                    in1=exo2[:].unsqueeze(2).broadcast_to([128, J, HC1]),
                    op=OP.mult)
                nc.vector.tensor_tensor(out=msh[:], in0=msh[:], in1=mo2[:],
                                        op=OP.add)

                rc2 = spool.tile([128, J], f32, tag="rc")
                nc.vector.tensor_scalar_add(out=rc2[:], in0=den2[:], scalar1=EPS)
                nc.vector.reciprocal(out=rc2[:], in_=rc2[:])

                z = spool.tile([128, J * C2], f32, tag="z")
                for j in range(J):
                    pst = ppool.tile([HC1, 128], f32, tag="pst")
                    nc.tensor.transpose(out=pst[:],
                                        in_=msh[:, j * HC1:(j + 1) * HC1],
                                        identity=idn[:])
                    mst = spool.tile([HC1, 128], f32, tag="mst")
                    nc.vector.tensor_copy(out=mst[:], in_=pst[:])
                    ps2 = ppool.tile([128, C2], f32, tag="ps2")
                    nc.tensor.matmul(ps2[:], lhsT=mst[:], rhs=w2s[:],
                                     start=True, stop=True)
                    # z = ps2 * (1/den) + b2
                    nc.vector.scalar_tensor_tensor(
                        out=z[:, j * C2:(j + 1) * C2],
                        in0=ps2[:], scalar=rc2[:, j:j + 1],
                        in1=b2s[:], op0=OP.mult, op1=OP.add)

                # log-softmax over the 40 classes
                zv = z[:].rearrange("p (j c) -> p j c", c=C2)
                mx = spool.tile([128, J], f32, tag="mx")
                nc.vector.tensor_reduce(out=mx[:], in_=zv, axis=AX.X, op=OP.max)
                sh = spool.tile([128, J * C2], f32, tag="sh")
                nc.vector.tensor_tensor(
                    out=sh[:].rearrange("p (j c) -> p j c", c=C2),
                    in0=zv,
                    in1=mx[:].unsqueeze(2).broadcast_to([128, J, C2]),
                    op=OP.subtract)
                ee = spool.tile([128, J * C2], f32, tag="ee")
                nc.scalar.activation(out=ee[:], in_=sh[:], func=ACTF.Exp)
                sm = spool.tile([128, J], f32, tag="sm")
                nc.vector.tensor_reduce(
                    out=sm[:], in_=ee[:].rearrange("p (j c) -> p j c", c=C2),
                    axis=AX.X, op=OP.add)
                lg = spool.tile([128, J], f32, tag="lg")
                nc.scalar.activation(out=lg[:], in_=sm[:], func=ACTF.Ln)
                oo = spool.tile([128, J * C2], f32, tag="oo")
                nc.vector.tensor_tensor(
                    out=oo[:].rearrange("p (j c) -> p j c", c=C2),
                    in0=sh[:].rearrange("p (j c) -> p j c", c=C2),
                    in1=lg[:].unsqueeze(2).broadcast_to([128, J, C2]),
                    op=OP.subtract)

                nc.sync.dma_start(
                    out=outp[t0 * 128:(t0 + J) * 128, :]
                        .rearrange("(j p) c -> p j c", p=128),
                    in_=oo[:].rearrange("p (j c) -> p j c", c=C2))

            if DEBUG_BUILD:
                nc.sync.dma_start(out=d_bnc1[:], in_=bnc1[:])
                nc.sync.dma_start(out=d_bnc2[:], in_=bnc2[:])
                nc.sync.dma_start(out=d_tab1[:], in_=tab1[:])

    nc.compile()
    return nc


# ---------------------------------------------------------------- entry
def kernel(x, edge_index, W1, att_src1, att_dst1, b1, W2, att_src2, att_dst2, b2,
           _debug_trace=False):
    from concourse.bass_utils import run_bass_kernel_spmd

    x = np.asarray(x)
    edge_index = np.asarray(edge_index)
    in_dtype = edge_index.dtype

    prep = _preprocess(edge_index)
    in_maps = _make_inputs(np.asarray(x, np.float32), np.asarray(W1, np.float32),
                           np.asarray(att_src1, np.float32),
                           np.asarray(att_dst1, np.float32),
                           np.asarray(b1, np.float32),
                           np.asarray(W2, np.float32),
                           np.asarray(att_src2, np.float32),
                           np.asarray(att_dst2, np.float32),
                           np.asarray(b2, np.float32), prep)
    nc = _build(prep)

    res = run_bass_kernel_spmd(nc, in_maps, list(range(NCORES)),
                               trace=_debug_trace)
    out = np.empty((N, C2), np.float32)
    for c in range(NCORES):
        out[prep["perms"][c]] = np.asarray(res.results[c]["out"])[:NPC]
    kernel._last_results = res
    return out

